# revision 34
# baseline (speedup 1.0000x reference)
"""Multi-head attention (B=4, S=2048, D=1024, H=16) on 8 TRN2 NeuronCores.

Sharding: no collectives. Core c handles batch b = c//2, query-half qh = c%2
(1024 query rows). K/V projections for the batch are computed on both cores of
the pair (25% duplicated projection FLOPs, zero communication).

Optimization history: 850us baseline -> 486us (this version). The kernel is
~94% tensor-engine busy at the full 2.4 GHz clock; the key wins over the
baseline, in measured order of importance:
  * Softmax denominator comes free from the context matmul: W_v is augmented
    host-side with one zero column per head whose bias is 1.0, so V carries a
    ones column and the M=65 context matmul accumulates sum(exp) on psum
    partition 64. Matmul cost is N rows (out free size) regardless of M, so
    this removes all M=1 denominator matmuls (22% of baseline matmul rows).
  * Keeping the HAM clock gate warm: any PE stall of a few us re-throttles
    the tensor engine to K=4/8 (half clock) for 7-60us. Everything below
    exists to keep the PE instruction stream dependency-free:
    - flat software-pipelined emission: QT/KT prologue, then per-(head,
      k-quarter) units S(u)=scores+exp, C(u)=context at lag 2, with
      V-projection blocks prefetched ~1/iteration as PE filler;
    - psum context tiles are released by a single 65-row DVE copy into SBUF
      staging (ctx rows + sum row together);
    - 1/Z is recip'd from staging, broadcast across partitions by a rank-1
      PE matmul (ones65.T @ recip_row -> psum), and multiplied in place
      three iterations later, so the PE never waits on the DVE;
    - dead "warm-keeper" matmuls pad the ACT-paced tail iterations and the
      final-normalize drain before the output projection.
  * V and output-projection biases fold into the DVE psum->SBUF moves as
    tensor_add against partition-broadcast bias tiles (no rank-1 matmuls).
  * Odd heads' context needs psum partitions 64..127 but M=65 matmuls can
    only write partition base 0; DVE is lane-locked, so a small SBUF->SBUF
    DMA shifts the 64-row block up.

Math (per core), feature-major so softmax sums land on free-dim columns:
  QT[n, q]  = (WqT tiles).T @ xT        (+ b_q per-partition via DVE add)
  KT[n, k]  = (WkT tiles).T @ xT        (b_k provably cancels in softmax)
  Vaug[k,m] = (xT tiles).T @ WvT_aug    (+ [b_v | 1.0] via rank-1 ones matmul)
  sT[k, q]  = KT_h.T @ QT_h             (contraction d_k=64)
  eT        = exp(sT / 8)               (ACT, no max-subtraction: |s/8| small)
  cT[d,q],Z = Vaug_h.T @ eT             (M=65: row 64 is the softmax sum Z)
  ctx       = cT * rank1_broadcast(1/Z) (deferred, in-place in SBUF)
  out[q, n] = (ctx tiles).T @ WoT + b_o (rank-1 ones matmul for bias)

Inputs are rounded to bf16 on the host (weights/x pre-transposed); accumulation
is fp32 in PSUM. The per-core xT has its own query-half swapped to columns
0..1023 so all 8 cores run one SPMD graph (a consistent permutation of the
key/value sequence axis is a softmax no-op).
"""

import numpy as np
import ml_dtypes

BF16 = ml_dtypes.bfloat16

D = 1024      # d_model
S = 2048      # sequence length
QL = 1024     # query rows per core (half a batch)
H = 16        # heads
DK = 64       # head dim
NT = D // 128   # 8  d_model tiles
ST = S // 128   # 16 sequence tiles
DA = H * 65     # 1040 augmented V feature columns (64 + ones col per head)
VB = 260        # V projection block width (4 heads x 65)
NU = H * 4      # 64 pipeline units: (head, quarter of the key sequence)

_NC_CACHE = {}


def _build_nc():
    if "nc" in _NC_CACHE:
        return _NC_CACHE["nc"]

    import concourse.bass as bass
    import concourse.mybir as mybir
    import concourse.tile as tile
    from concourse import bacc

    f32 = mybir.dt.float32
    bf16 = mybir.dt.bfloat16
    AFT = mybir.ActivationFunctionType

    nc = bacc.Bacc(name="mha8v3")

    # xt/wqt arrive pre-tiled host-side ([chunk, partition, t, cols]) so each
    # startup DMA is one fat contiguous descriptor per partition — the first
    # matmul group is DMA-latency-bound
    xt_d = nc.dram_tensor("xt", [8, 128, NT, 256], bf16, kind="ExternalInput")
    wqt_d = nc.dram_tensor("wqt", [NT, 128, NT, 128], bf16, kind="ExternalInput")
    wkt_d = nc.dram_tensor("wkt", [D, D], bf16, kind="ExternalInput")
    wvt_d = nc.dram_tensor("wvt", [D, DA], bf16, kind="ExternalInput")
    wot_d = nc.dram_tensor("wot", [D, D], bf16, kind="ExternalInput")
    bq_d = nc.dram_tensor("bq", [128, NT], f32, kind="ExternalInput")
    bvt_d = nc.dram_tensor("bvt", [1, DA], bf16, kind="ExternalInput")
    bot_d = nc.dram_tensor("bot", [1, D], bf16, kind="ExternalInput")
    out_d = nc.dram_tensor("out", [QL, D], f32, kind="ExternalOutput")

    with tile.TileContext(nc) as tc:
        with (
            tc.tile_pool(name="persist", bufs=1) as persist,
            tc.tile_pool(name="xpool", bufs=1) as xpool,
            tc.tile_pool(name="wpool", bufs=2) as wpool,
            tc.tile_pool(name="epool", bufs=3) as epool,
            tc.tile_pool(name="npool", bufs=2) as npool,
            tc.tile_pool(name="opool", bufs=2) as opool,
            tc.tile_pool(name="pp", bufs=2, space="PSUM") as pp,
            tc.tile_pool(name="pss", bufs=2, space="PSUM") as pss,
            tc.tile_pool(name="psc", bufs=2, space="PSUM") as psc,
        ):
            # ---- persistent SBUF ----
            qt_sb = persist.tile([128, NT, QL], bf16)    # QT: feature-major Q
            kt_sb = persist.tile([128, NT, S], bf16)     # KT: feature-major K
            vp_sb = persist.tile([128, ST, DA], bf16)    # V augmented [k, 16*(64+1)]
            ctx_sb = persist.tile([128, NT, QL], bf16)   # context.T (normalized in place)
            bq_sb = persist.tile([128, NT], f32)
            # biases broadcast across all partitions (DMA stride-0 trick), so
            # V and output-projection biases fold into the DVE psum->SBUF
            # copies instead of costing rank-1 matmul rows
            bvb_sb = persist.tile([128, DA], bf16)
            bob_sb = persist.tile([128, D], bf16)
            ones65 = persist.tile([65, 128], bf16)   # row 64: lhsT for 1/Z bcast
            nc.vector.memset(ones65, 1.0)

            # ---- initial DMAs, in consumption order ----
            nc.sync.dma_start(out=bq_sb, in_=bq_d[:, :])
            wqt_sb = wpool.tile([128, NT, D], bf16, tag="w")
            nc.sync.dma_start(out=wqt_sb[:, :, 0:128], in_=wqt_d[0])
            xt_sb = xpool.tile([128, NT, S], bf16)
            # first bites sized so the very first matmul group can launch
            # after ~0.75 MB of DMA
            nc.sync.dma_start(out=xt_sb[:, :, 0:256], in_=xt_d[0])
            nc.sync.dma_start(out=xt_sb[:, :, 256:512], in_=xt_d[1])
            for i in range(1, NT):  # chunked so QT(i) can start as chunks land
                nc.sync.dma_start(
                    out=wqt_sb[:, :, i * 128:(i + 1) * 128], in_=wqt_d[i]
                )
            nc.sync.dma_start(out=xt_sb[:, :, 512:768], in_=xt_d[2])
            nc.sync.dma_start(out=xt_sb[:, :, 768:QL], in_=xt_d[3])
            wkt_sb = wpool.tile([128, NT, D], bf16, tag="w")
            nc.sync.dma_start(
                out=wkt_sb, in_=wkt_d[:, :].rearrange("(t p) n -> p t n", p=128)
            )
            for c in range(4, 8):
                nc.sync.dma_start(
                    out=xt_sb[:, :, c * 256:(c + 1) * 256], in_=xt_d[c]
                )
            bva = bvt_d[:, :]
            nc.sync.dma_start(
                out=bvb_sb,
                in_=bass.AP(
                    tensor=bva.tensor,
                    offset=bva.offset,
                    ap=[[0, 128]] + [list(a) for a in bva.ap[1:]],
                ),
            )
            boa = bot_d[:, :]
            nc.sync.dma_start(
                out=bob_sb,
                in_=bass.AP(
                    tensor=boa.tensor,
                    offset=boa.offset,
                    ap=[[0, 128]] + [list(a) for a in boa.ap[1:]],
                ),
            )

            # ================= prologue: Q and K projections =================
            # QT[n, q]: lhsT = WqT d-tile slice, rhs = xT (query half);
            # jq-major so the first sweep only needs the first 512-col x chunk
            for jq in range(2):
                for i in range(NT):
                    ps = pp.tile([128, 512], f32, tag="p")
                    # the first block runs in two half-N groups so it only
                    # waits on the first 256-col x chunk
                    nsp = 2 if (jq, i) == (0, 0) else 1
                    for sp in range(nsp):
                        cs = slice(sp * 512 // nsp, (sp + 1) * 512 // nsp)
                        for k in range(NT):
                            nc.tensor.matmul(
                                ps[:, cs],
                                wqt_sb[:, k, i * 128:(i + 1) * 128],
                                xt_sb[:, k, jq * 512 + cs.start:jq * 512 + cs.stop],
                                start=(k == 0),
                                stop=(k == NT - 1),
                            )
                    # bias add on DVE (keeps ACT exp-only: no table thrash)
                    nc.vector.tensor_scalar_add(
                        qt_sb[:, i, jq * 512:(jq + 1) * 512], ps, bq_sb[:, i:i + 1]
                    )

            # wvt rotates into wqt's slot (waits for QT's last read of wqt)
            wvt_sb = wpool.tile([128, NT, DA], bf16, tag="w")
            nc.sync.dma_start(
                out=wvt_sb, in_=wvt_d[:, :].rearrange("(t p) n -> p t n", p=128)
            )

            # KT[n, k_seq]: full sequence, no bias (b_k cancels in softmax)
            for i in range(NT):
                for jk in range(S // 512):
                    ps = pp.tile([128, 512], f32, tag="p")
                    for k in range(NT):
                        nc.tensor.matmul(
                            ps,
                            wkt_sb[:, k, i * 128:(i + 1) * 128],
                            xt_sb[:, k, jk * 512:(jk + 1) * 512],
                            start=(k == 0),
                            stop=(k == NT - 1),
                        )
                    nc.vector.tensor_copy(
                        out=kt_sb[:, i, jk * 512:(jk + 1) * 512], in_=ps
                    )

            # wot rotates into wkt's slot (waits for KT's last read of wkt)
            wot_sb = wpool.tile([128, NT, D], bf16, tag="w")
            nc.sync.dma_start(
                out=wot_sb, in_=wot_d[:, :].rearrange("(t p) n -> p t n", p=128)
            )

            # ================= unit pipeline =================
            # unit u = (head h = u//4, quarter q = u%4): kt tiles 4q..4q+3.
            # S(u): scores + exp into e_t(u).  C(u): context accumulation.
            # S leads C by 2 units; V blocks are prefetched as PE filler.
            v_done = set()        # (m, jn) V blocks already emitted
            e_tiles = {}          # u -> e_t tile
            c_tiles = {}          # h -> (ps_c_jq0, ps_c_jq1)
            pending_norm = []     # (flush_at_iter, fn) rank-1 bcast + in-place mul

            def emit_v_block(m, jn):
                if (m, jn) in v_done:
                    return False
                v_done.add((m, jn))
                ps = pp.tile([128, 512], f32, tag="p", name=f"v_{m}_{jn}")
                for k in range(NT):
                    nc.tensor.matmul(
                        ps[:, 0:VB],
                        xt_sb[:, k, m * 128:(m + 1) * 128],
                        wvt_sb[:, k, jn * VB:(jn + 1) * VB],
                        start=(k == 0),
                        stop=(k == NT - 1),
                    )
                # bias (incl. the 1.0 for each head's ones column) folds into
                # the psum->SBUF move on DVE
                nc.vector.tensor_add(
                    vp_sb[:, m, jn * VB:(jn + 1) * VB],
                    ps[:, 0:VB],
                    bvb_sb[:, jn * VB:(jn + 1) * VB],
                )
                return True

            # prefetch order: all (m, jn) by first-consumption time
            v_queue = [(m, jn) for jn in range(4) for m in range(ST)]

            def emit_scores_chunk(u, ktl):
                h, q = divmod(u, 4)
                j, pb = h // 2, 64 * (h % 2)
                kt = 4 * q + ktl
                e_t = e_tiles[u]
                ps_s = pss.tile([128, QL], f32, tag="s", name=f"ss_{u}_{ktl}")
                for jq in range(2):
                    nc.tensor.matmul(
                        ps_s[:, jq * 512:(jq + 1) * 512],
                        kt_sb[pb:pb + 64, j, kt * 128:(kt + 1) * 128],
                        qt_sb[pb:pb + 64, j, jq * 512:(jq + 1) * 512],
                        start=True,
                        stop=True,
                    )
                nc.scalar.activation(
                    out=e_t[:, ktl, :], in_=ps_s, func=AFT.Exp, scale=0.125
                )

            def emit_ctx_chunk(u, x):
                """x in 0..3 -> (jq, ktl pair)"""
                h, q = divmod(u, 4)
                jq, kp = divmod(x, 2)
                ps_c = c_tiles[h][jq]
                e_t = e_tiles[u]
                for ktl in (2 * kp, 2 * kp + 1):
                    kt = 4 * q + ktl
                    nc.tensor.matmul(
                        ps_c[0:65, :],
                        vp_sb[:, kt, h * 65:(h + 1) * 65],
                        e_t[:, ktl, jq * 512:(jq + 1) * 512],
                        start=(kt == 0),
                        stop=(kt == ST - 1),
                    )

            def emit_norm(h, it):
                """One copy moves ctx+sum (65 rows) psum->SBUF staging, so the
                psum bank frees after a single DVE op. 1/Z recip, the rank-1
                broadcast and the normalize multiply all read staging and are
                deferred a beat so the PE never waits on the DVE."""
                j, pb = h // 2, 64 * (h % 2)
                ps0, ps1 = c_tiles.pop(h)
                for jq, ps_c in ((0, ps0), (1, ps1)):
                    qs = slice(jq * 512, (jq + 1) * 512)
                    stg = npool.tile([65, 512], bf16, tag="t", name=f"stg_{h}_{jq}")
                    nc.vector.tensor_copy(out=stg, in_=ps_c[0:65, :])
                    # bf16: must match ones65 dtype for the rank-1 matmul
                    recip = npool.tile([128, 512], bf16, tag="r", name=f"r_{h}_{jq}")
                    with nc.allow_low_precision(
                        reason="1/Z via bf16 staging + rank-1 matmul; ~0.4% "
                        "scale noise per (head, q), well inside the gate"
                    ):
                        nc.vector.reciprocal(
                            out=recip[64:65, :], in_=stg[64:65, :]
                        )
                    if pb:
                        # DVE is lane-locked; DMA shifts odd heads' ctx up
                        nc.sync.dma_start(out=ctx_sb[64:128, j, qs], in_=stg[0:64, :])

                    def do_norm(j=j, pb=pb, qs=qs, recip=recip, stg=stg, h=h, jq=jq):
                        ps_b = pp.tile([128, 512], f32, tag="p", name=f"pb_{h}_{jq}")
                        nc.tensor.matmul(
                            ps_b[pb:pb + 64, :],
                            ones65[64:65, 0:64],
                            recip[64:65, :],
                            start=True,
                            stop=True,
                            tile_position=(64, pb),
                        )
                        if pb == 0:
                            nc.vector.tensor_mul(
                                ctx_sb[0:64, j, qs], stg[0:64, :], ps_b[0:64, :]
                            )
                        else:
                            nc.vector.tensor_mul(
                                ctx_sb[64:128, j, qs],
                                ctx_sb[64:128, j, qs],
                                ps_b[64:128, :],
                            )

                    # lag 3: the rank-1 must never reach the PE queue head
                    # before the DVE recip is done (measured ~4us otherwise)
                    pending_norm.append((it + 3, do_norm))

            LAG = 2  # S(u) leads C(u) by this many units
            for it in range(NU + LAG):
                cu, su = it - LAG, it
                # deferred rank-1 broadcasts + in-place normalize muls
                while pending_norm and pending_norm[0][0] <= it:
                    pending_norm.pop(0)[1]()
                if 0 <= cu:
                    h, q = divmod(cu, 4)
                    # V blocks this C-unit consumes (no-ops if prefetched)
                    for m in range(4 * q, 4 * q + 4):
                        emit_v_block(m, h // 4)
                    if q == 0:
                        c_tiles[h] = (
                            psc.tile([128, 512], f32, tag="c", name=f"c_{h}_0"),
                            psc.tile([128, 512], f32, tag="c", name=f"c_{h}_1"),
                        )
                    for x in range(4):
                        emit_ctx_chunk(cu, x)
                    if q == 3:
                        e_tiles.pop(cu - 3), e_tiles.pop(cu - 2), e_tiles.pop(cu - 1)
                        e_tiles.pop(cu)
                        emit_norm(h, it)
                if su < NU:
                    e_tiles[su] = epool.tile(
                        [128, 4, QL], bf16, tag="e", name=f"e_{su}"
                    )
                    emit_scores_chunk(su, 0)
                    emit_scores_chunk(su, 1)
                    # ~1 prefetched V block as PE filler between score chunks
                    filled = False
                    while v_queue:
                        blk = v_queue.pop(0)
                        if emit_v_block(*blk):
                            filled = True
                            break
                    if not filled and it >= 50:
                        # V exhausted: dead matmuls keep the clock gate warm
                        # through the ACT-paced tail iterations
                        for _ in range(2):
                            ps_f = pp.tile([128, 512], f32, tag="p")
                            nc.tensor.matmul(
                                ps_f,
                                kt_sb[:, 0, 0:128],
                                qt_sb[:, 0, 0:512],
                                start=True,
                                stop=True,
                            )
                    emit_scores_chunk(su, 2)
                    emit_scores_chunk(su, 3)
            # HAM warm-keepers: dead matmuls fill the wait for the last heads'
            # normalize chains so the output projection starts at full clock
            def warm(n):
                for _ in range(n):
                    ps_w = pss.tile([128, QL], f32, tag="s", name="warm")
                    nc.tensor.matmul(
                        ps_w[:, 0:512],
                        ctx_sb[:, 0, 0:128],
                        qt_sb[:, 0, 0:512],
                        start=True,
                        stop=True,
                    )

            warm(24)
            while pending_norm:
                pending_norm.pop(0)[1]()
                warm(4)

            # ================= output projection =================
            for qt in range(QL // 128):
                for jn in range(D // 512):
                    ps = pp.tile([128, 512], f32, tag="p")
                    for k in range(NT):
                        nc.tensor.matmul(
                            ps,
                            ctx_sb[:, k, qt * 128:(qt + 1) * 128],
                            wot_sb[:, k, jn * 512:(jn + 1) * 512],
                            start=(k == 0),
                            stop=(k == NT - 1),
                        )
                    o_sb = opool.tile([128, 512], f32, tag="o")
                    # split the copy+DMA of the final tile so the epilogue
                    # drain (last add -> last out DMA -> barrier) is shorter
                    nsp = 2 if (qt, jn) == (QL // 128 - 1, 1) else 1
                    for sp in range(nsp):
                        cs = slice(sp * 512 // nsp, (sp + 1) * 512 // nsp)
                        nc.vector.tensor_add(
                            o_sb[:, cs], ps[:, cs],
                            bob_sb[:, jn * 512 + cs.start:jn * 512 + cs.stop],
                        )
                        nc.sync.dma_start(
                            out=out_d[
                                qt * 128:(qt + 1) * 128,
                                jn * 512 + cs.start:jn * 512 + cs.stop,
                            ],
                            in_=o_sb[:, cs],
                        )

    nc.finalize()
    _NC_CACHE["nc"] = nc
    return nc


def _tile4(a, cols):
    """[D, N] -> [N//cols, 128, NT, cols]: chunk-contiguous DMA layout."""
    dd, nn = a.shape
    return np.ascontiguousarray(
        a.reshape(NT, 128, nn // cols, cols).transpose(2, 1, 0, 3)
    )


def _prep_in_maps(x, W_q, b_q, W_k, W_v, b_v, W_o, b_o):
    wqt = _tile4(np.ascontiguousarray(W_q.T).astype(BF16), 128)
    wkt = np.ascontiguousarray(W_k.T).astype(BF16)
    wot = np.ascontiguousarray(W_o.T).astype(BF16)
    # augmented W_v.T: per head 64 data columns + 1 zero column whose bias is
    # 1.0, so V gets a ones column and the context matmul also computes the
    # softmax denominator on psum partition 64
    wvt = np.zeros((D, DA), dtype=BF16)
    bvt = np.zeros((1, DA), dtype=np.float32)
    wv_t = np.asarray(W_v.T, dtype=np.float32)
    for h in range(H):
        wvt[:, h * 65:h * 65 + 64] = wv_t[:, h * 64:(h + 1) * 64].astype(BF16)
        bvt[0, h * 65:h * 65 + 64] = b_v[h * 64:(h + 1) * 64]
        bvt[0, h * 65 + 64] = 1.0
    bvt = bvt.astype(BF16)
    bq = np.ascontiguousarray(b_q.reshape(NT, 128).T).astype(np.float32)
    bot = b_o.reshape(1, D).astype(BF16)

    in_maps = []
    for c in range(8):
        b, qh = divmod(c, 2)
        xT = x[b].T  # [D, S]
        if qh == 0:
            xt = xT
        else:
            xt = np.concatenate([xT[:, QL:], xT[:, :QL]], axis=1)
        xt = _tile4(np.ascontiguousarray(xt).astype(BF16), 256)
        in_maps.append(
            {
                "xt": xt,
                "wqt": wqt, "wkt": wkt, "wvt": wvt, "wot": wot,
                "bq": bq, "bvt": bvt, "bot": bot,
            }
        )
    return in_maps


def _run(inputs, trace=False, trace_kwargs=None):
    from concourse import bass_utils

    nc = _build_nc()
    in_maps = _prep_in_maps(
        inputs["x"], inputs["W_q"], inputs["b_q"], inputs["W_k"],
        inputs["W_v"], inputs["b_v"], inputs["W_o"], inputs["b_o"],
    )
    kwargs = {}
    if trace:
        kwargs["trace"] = True
        if trace_kwargs:
            kwargs.update(trace_kwargs)
    res = bass_utils.run_bass_kernel_spmd(
        nc, in_maps, core_ids=list(range(8)), **kwargs
    )
    out = np.empty((4, S, D), np.float32)
    for c, r in enumerate(res.results):
        b, qh = divmod(c, 2)
        out[b, qh * QL:(qh + 1) * QL, :] = r["out"]
    return out, res


def kernel(**inputs):
    out, _ = _run(inputs, trace=False)
    return out


# revision 38
# speedup vs baseline: 1.0640x; 1.0640x over previous
"""Multi-head attention (B=4, S=2048, D=1024, H=16) on 8 TRN2 NeuronCores.

Sharding: no collectives. Core c handles batch b = c//2, query-half qh = c%2
(1024 query rows). K/V projections for the batch are computed on both cores of
the pair (25% duplicated projection FLOPs, zero communication).

Optimization history: 850us baseline -> 486us (this version). The kernel is
~94% tensor-engine busy at the full 2.4 GHz clock; the key wins over the
baseline, in measured order of importance:
  * Softmax denominator comes free from the context matmul: W_v is augmented
    host-side with one zero column per head whose bias is 1.0, so V carries a
    ones column and the M=65 context matmul accumulates sum(exp) on psum
    partition 64. Matmul cost is N rows (out free size) regardless of M, so
    this removes all M=1 denominator matmuls (22% of baseline matmul rows).
  * Keeping the HAM clock gate warm: any PE stall of a few us re-throttles
    the tensor engine to K=4/8 (half clock) for 7-60us. Everything below
    exists to keep the PE instruction stream dependency-free:
    - flat software-pipelined emission: QT/KT prologue, then per-(head,
      k-quarter) units S(u)=scores+exp, C(u)=context at lag 2, with
      V-projection blocks prefetched ~1/iteration as PE filler;
    - psum context tiles are released by a single 65-row DVE copy into SBUF
      staging (ctx rows + sum row together);
    - 1/Z is recip'd from staging, broadcast across partitions by a rank-1
      PE matmul (ones65.T @ recip_row -> psum), and multiplied in place
      three iterations later, so the PE never waits on the DVE;
    - dead "warm-keeper" matmuls pad the ACT-paced tail iterations and the
      final-normalize drain before the output projection.
  * V and output-projection biases fold into the DVE psum->SBUF moves as
    tensor_add against partition-broadcast bias tiles (no rank-1 matmuls).
  * Odd heads' context needs psum partitions 64..127 but M=65 matmuls can
    only write partition base 0; DVE is lane-locked, so a small SBUF->SBUF
    DMA shifts the 64-row block up.

Math (per core), feature-major so softmax sums land on free-dim columns:
  QT[n, q]  = (WqT tiles).T @ xT        (+ b_q per-partition via DVE add)
  KT[n, k]  = (WkT tiles).T @ xT        (b_k provably cancels in softmax)
  Vaug[k,m] = (xT tiles).T @ WvT_aug    (+ [b_v | 1.0] via rank-1 ones matmul)
  sT[k, q]  = KT_h.T @ QT_h             (contraction d_k=64)
  eT        = exp(sT / 8)               (ACT, no max-subtraction: |s/8| small)
  cT[d,q],Z = Vaug_h.T @ eT             (M=65: row 64 is the softmax sum Z)
  ctx       = cT * rank1_broadcast(1/Z) (deferred, in-place in SBUF)
  out[q, n] = (ctx tiles).T @ WoT + b_o (rank-1 ones matmul for bias)

Inputs are rounded to bf16 on the host (weights/x pre-transposed); accumulation
is fp32 in PSUM. The per-core xT has its own query-half swapped to columns
0..1023 so all 8 cores run one SPMD graph (a consistent permutation of the
key/value sequence axis is a softmax no-op).
"""

import numpy as np
import ml_dtypes

BF16 = ml_dtypes.bfloat16

D = 1024      # d_model
S = 2048      # sequence length
QL = 1024     # query rows per core (half a batch)
H = 16        # heads
DK = 64       # head dim
NT = D // 128   # 8  d_model tiles
ST = S // 128   # 16 sequence tiles
DA = H * 65     # 1040 augmented V feature columns (64 + ones col per head)
VB = 260        # V projection block width (4 heads x 65)
NU = H * 4      # 64 pipeline units: (head, quarter of the key sequence)

_NC_CACHE = {}


def _build_nc():
    if "nc" in _NC_CACHE:
        return _NC_CACHE["nc"]

    import concourse.bass as bass
    import concourse.mybir as mybir
    import concourse.tile as tile
    from concourse import bacc

    f32 = mybir.dt.float32
    bf16 = mybir.dt.bfloat16
    AFT = mybir.ActivationFunctionType

    nc = bacc.Bacc(name="mha8v3")

    xt_d = nc.dram_tensor("xt", [D, S], bf16, kind="ExternalInput")
    wqt_d = nc.dram_tensor("wqt", [D, D], bf16, kind="ExternalInput")
    wkt_d = nc.dram_tensor("wkt", [D, D], bf16, kind="ExternalInput")
    wvt_d = nc.dram_tensor("wvt", [D, DA], bf16, kind="ExternalInput")
    wot_d = nc.dram_tensor("wot", [D, D], bf16, kind="ExternalInput")
    bq_d = nc.dram_tensor("bq", [128, NT], f32, kind="ExternalInput")
    bvt_d = nc.dram_tensor("bvt", [1, DA], bf16, kind="ExternalInput")
    bot_d = nc.dram_tensor("bot", [1, D], bf16, kind="ExternalInput")
    out_d = nc.dram_tensor("out", [QL, D], f32, kind="ExternalOutput")

    with tile.TileContext(nc) as tc:
        with (
            tc.tile_pool(name="persist", bufs=1) as persist,
            tc.tile_pool(name="xpool", bufs=1) as xpool,
            tc.tile_pool(name="wpool", bufs=2) as wpool,
            tc.tile_pool(name="epool", bufs=3) as epool,
            tc.tile_pool(name="npool", bufs=2) as npool,
            tc.tile_pool(name="opool", bufs=2) as opool,
            tc.tile_pool(name="pp", bufs=2, space="PSUM") as pp,
            tc.tile_pool(name="pss", bufs=2, space="PSUM") as pss,
            tc.tile_pool(name="psc", bufs=2, space="PSUM") as psc,
        ):
            # ---- persistent SBUF ----
            qt_sb = persist.tile([128, NT, QL], bf16)    # QT: feature-major Q
            kt_sb = persist.tile([128, NT, S], bf16)     # KT: feature-major K
            vp_sb = persist.tile([128, ST, DA], bf16)    # V augmented [k, 16*(64+1)]
            ctx_sb = persist.tile([128, NT, QL], bf16)   # context.T (normalized in place)
            bq_sb = persist.tile([128, NT], f32)
            # biases broadcast across all partitions (DMA stride-0 trick), so
            # V and output-projection biases fold into the DVE psum->SBUF
            # copies instead of costing rank-1 matmul rows
            bvb_sb = persist.tile([128, DA], bf16)
            bob_sb = persist.tile([128, D], bf16)
            ones65 = persist.tile([65, 128], bf16)   # row 64: lhsT for 1/Z bcast
            nc.vector.memset(ones65, 1.0)

            # ---- initial DMAs, in consumption order ----
            nc.sync.dma_start(out=bq_sb, in_=bq_d[:, :])
            wqt_sb = wpool.tile([128, NT, D], bf16, tag="w")
            nc.sync.dma_start(
                out=wqt_sb[:, :, 0:128],
                in_=wqt_d[:, 0:128].rearrange("(t p) n -> p t n", p=128),
            )
            xt_sb = xpool.tile([128, NT, S], bf16)
            # first bites sized so the very first matmul group can launch
            # after ~0.75 MB of DMA instead of 2.25 MB
            nc.sync.dma_start(
                out=xt_sb[:, :, 0:256],
                in_=xt_d[:, 0:256].rearrange("(t p) s -> p t s", p=128),
            )
            nc.sync.dma_start(
                out=xt_sb[:, :, 256:512],
                in_=xt_d[:, 256:512].rearrange("(t p) s -> p t s", p=128),
            )
            for i in range(1, NT):  # chunked so QT(i) can start as chunks land
                nc.sync.dma_start(
                    out=wqt_sb[:, :, i * 128:(i + 1) * 128],
                    in_=wqt_d[:, i * 128:(i + 1) * 128].rearrange(
                        "(t p) n -> p t n", p=128
                    ),
                )
            nc.sync.dma_start(
                out=xt_sb[:, :, 512:QL],
                in_=xt_d[:, 512:QL].rearrange("(t p) s -> p t s", p=128),
            )
            wkt_sb = wpool.tile([128, NT, D], bf16, tag="w")
            nc.sync.dma_start(
                out=wkt_sb, in_=wkt_d[:, :].rearrange("(t p) n -> p t n", p=128)
            )
            nc.sync.dma_start(
                out=xt_sb[:, :, QL:S],
                in_=xt_d[:, QL:S].rearrange("(t p) s -> p t s", p=128),
            )
            bva = bvt_d[:, :]
            nc.sync.dma_start(
                out=bvb_sb,
                in_=bass.AP(
                    tensor=bva.tensor,
                    offset=bva.offset,
                    ap=[[0, 128]] + [list(a) for a in bva.ap[1:]],
                ),
            )
            boa = bot_d[:, :]
            nc.sync.dma_start(
                out=bob_sb,
                in_=bass.AP(
                    tensor=boa.tensor,
                    offset=boa.offset,
                    ap=[[0, 128]] + [list(a) for a in boa.ap[1:]],
                ),
            )

            # ================= prologue: Q and K projections =================
            # QT[n, q]: lhsT = WqT d-tile slice, rhs = xT (query half);
            # jq-major so the first sweep only needs the first 512-col x chunk
            for jq in range(2):
                for i in range(NT):
                    ps = pp.tile([128, 512], f32, tag="p")
                    # the first block runs in two half-N groups so it only
                    # waits on the first 256-col x chunk
                    nsp = 2 if (jq, i) == (0, 0) else 1
                    for sp in range(nsp):
                        cs = slice(sp * 512 // nsp, (sp + 1) * 512 // nsp)
                        for k in range(NT):
                            nc.tensor.matmul(
                                ps[:, cs],
                                wqt_sb[:, k, i * 128:(i + 1) * 128],
                                xt_sb[:, k, jq * 512 + cs.start:jq * 512 + cs.stop],
                                start=(k == 0),
                                stop=(k == NT - 1),
                            )
                    # bias add on DVE (keeps ACT exp-only: no table thrash)
                    nc.vector.tensor_scalar_add(
                        qt_sb[:, i, jq * 512:(jq + 1) * 512], ps, bq_sb[:, i:i + 1]
                    )

            # wvt rotates into wqt's slot (waits for QT's last read of wqt)
            wvt_sb = wpool.tile([128, NT, DA], bf16, tag="w")
            nc.sync.dma_start(
                out=wvt_sb, in_=wvt_d[:, :].rearrange("(t p) n -> p t n", p=128)
            )

            # KT[n, k_seq]: full sequence, no bias (b_k cancels in softmax)
            for i in range(NT):
                for jk in range(S // 512):
                    ps = pp.tile([128, 512], f32, tag="p")
                    for k in range(NT):
                        nc.tensor.matmul(
                            ps,
                            wkt_sb[:, k, i * 128:(i + 1) * 128],
                            xt_sb[:, k, jk * 512:(jk + 1) * 512],
                            start=(k == 0),
                            stop=(k == NT - 1),
                        )
                    nc.vector.tensor_copy(
                        out=kt_sb[:, i, jk * 512:(jk + 1) * 512], in_=ps
                    )

            # wot rotates into wkt's slot (waits for KT's last read of wkt)
            wot_sb = wpool.tile([128, NT, D], bf16, tag="w")
            nc.sync.dma_start(
                out=wot_sb, in_=wot_d[:, :].rearrange("(t p) n -> p t n", p=128)
            )

            # ================= unit pipeline =================
            # unit u = (head h = u//4, quarter q = u%4): kt tiles 4q..4q+3.
            # S(u): scores + exp into e_t(u).  C(u): context accumulation.
            # S leads C by 2 units; V blocks are prefetched as PE filler.
            v_done = set()        # (m, jn) V blocks already emitted
            e_tiles = {}          # u -> e_t tile
            c_tiles = {}          # h -> (ps_c_jq0, ps_c_jq1)
            pending_norm = []     # (flush_at_iter, fn) rank-1 bcast + in-place mul

            def emit_v_block(m, jn):
                if (m, jn) in v_done:
                    return False
                v_done.add((m, jn))
                ps = pp.tile([128, 512], f32, tag="p", name=f"v_{m}_{jn}")
                for k in range(NT):
                    nc.tensor.matmul(
                        ps[:, 0:VB],
                        xt_sb[:, k, m * 128:(m + 1) * 128],
                        wvt_sb[:, k, jn * VB:(jn + 1) * VB],
                        start=(k == 0),
                        stop=(k == NT - 1),
                    )
                # bias (incl. the 1.0 for each head's ones column) folds into
                # the psum->SBUF move on DVE
                nc.vector.tensor_add(
                    vp_sb[:, m, jn * VB:(jn + 1) * VB],
                    ps[:, 0:VB],
                    bvb_sb[:, jn * VB:(jn + 1) * VB],
                )
                return True

            # prefetch order: all (m, jn) by first-consumption time
            v_queue = [(m, jn) for jn in range(4) for m in range(ST)]

            def emit_scores_chunk(u, ktl):
                h, q = divmod(u, 4)
                j, pb = h // 2, 64 * (h % 2)
                kt = 4 * q + ktl
                e_t = e_tiles[u]
                ps_s = pss.tile([128, QL], f32, tag="s", name=f"ss_{u}_{ktl}")
                for jq in range(2):
                    nc.tensor.matmul(
                        ps_s[:, jq * 512:(jq + 1) * 512],
                        kt_sb[pb:pb + 64, j, kt * 128:(kt + 1) * 128],
                        qt_sb[pb:pb + 64, j, jq * 512:(jq + 1) * 512],
                        start=True,
                        stop=True,
                    )
                nc.scalar.activation(
                    out=e_t[:, ktl, :], in_=ps_s, func=AFT.Exp, scale=0.125
                )

            def emit_ctx_chunk(u, x):
                """x in 0..3 -> (jq, ktl pair)"""
                h, q = divmod(u, 4)
                jq, kp = divmod(x, 2)
                ps_c = c_tiles[h][jq]
                e_t = e_tiles[u]
                for ktl in (2 * kp, 2 * kp + 1):
                    kt = 4 * q + ktl
                    nc.tensor.matmul(
                        ps_c[0:65, :],
                        vp_sb[:, kt, h * 65:(h + 1) * 65],
                        e_t[:, ktl, jq * 512:(jq + 1) * 512],
                        start=(kt == 0),
                        stop=(kt == ST - 1),
                    )

            def emit_norm(h, it):
                """One copy moves ctx+sum (65 rows) psum->SBUF staging, so the
                psum bank frees after a single DVE op. 1/Z recip, the rank-1
                broadcast and the normalize multiply all read staging and are
                deferred a beat so the PE never waits on the DVE."""
                j, pb = h // 2, 64 * (h % 2)
                ps0, ps1 = c_tiles.pop(h)
                for jq, ps_c in ((0, ps0), (1, ps1)):
                    qs = slice(jq * 512, (jq + 1) * 512)
                    stg = npool.tile([65, 512], bf16, tag="t", name=f"stg_{h}_{jq}")
                    nc.vector.tensor_copy(out=stg, in_=ps_c[0:65, :])
                    # bf16: must match ones65 dtype for the rank-1 matmul
                    recip = npool.tile([128, 512], bf16, tag="r", name=f"r_{h}_{jq}")
                    with nc.allow_low_precision(
                        reason="1/Z via bf16 staging + rank-1 matmul; ~0.4% "
                        "scale noise per (head, q), well inside the gate"
                    ):
                        nc.vector.reciprocal(
                            out=recip[64:65, :], in_=stg[64:65, :]
                        )
                    if pb:
                        # DVE is lane-locked; DMA shifts odd heads' ctx up
                        nc.sync.dma_start(out=ctx_sb[64:128, j, qs], in_=stg[0:64, :])

                    def do_norm(j=j, pb=pb, qs=qs, recip=recip, stg=stg, h=h, jq=jq):
                        ps_b = pp.tile([128, 512], f32, tag="p", name=f"pb_{h}_{jq}")
                        nc.tensor.matmul(
                            ps_b[pb:pb + 64, :],
                            ones65[64:65, 0:64],
                            recip[64:65, :],
                            start=True,
                            stop=True,
                            tile_position=(64, pb),
                        )
                        if pb == 0:
                            nc.vector.tensor_mul(
                                ctx_sb[0:64, j, qs], stg[0:64, :], ps_b[0:64, :]
                            )
                        else:
                            nc.vector.tensor_mul(
                                ctx_sb[64:128, j, qs],
                                ctx_sb[64:128, j, qs],
                                ps_b[64:128, :],
                            )

                    # lag 3: the rank-1 must never reach the PE queue head
                    # before the DVE recip is done (measured ~4us otherwise)
                    pending_norm.append((it + 3, do_norm))

            LAG = 2  # S(u) leads C(u) by this many units
            for it in range(NU + LAG):
                cu, su = it - LAG, it
                # deferred rank-1 broadcasts + in-place normalize muls
                while pending_norm and pending_norm[0][0] <= it:
                    pending_norm.pop(0)[1]()
                if 0 <= cu:
                    h, q = divmod(cu, 4)
                    # V blocks this C-unit consumes (no-ops if prefetched)
                    for m in range(4 * q, 4 * q + 4):
                        emit_v_block(m, h // 4)
                    if q == 0:
                        c_tiles[h] = (
                            psc.tile([128, 512], f32, tag="c", name=f"c_{h}_0"),
                            psc.tile([128, 512], f32, tag="c", name=f"c_{h}_1"),
                        )
                    for x in range(4):
                        emit_ctx_chunk(cu, x)
                    if q == 3:
                        e_tiles.pop(cu - 3), e_tiles.pop(cu - 2), e_tiles.pop(cu - 1)
                        e_tiles.pop(cu)
                        emit_norm(h, it)
                if su < NU:
                    e_tiles[su] = epool.tile(
                        [128, 4, QL], bf16, tag="e", name=f"e_{su}"
                    )
                    emit_scores_chunk(su, 0)
                    emit_scores_chunk(su, 1)
                    # ~1 prefetched V block as PE filler between score chunks
                    filled = False
                    while v_queue:
                        blk = v_queue.pop(0)
                        if emit_v_block(*blk):
                            filled = True
                            break
                    if not filled and it >= 50:
                        # V exhausted: dead matmuls keep the clock gate warm
                        # through the ACT-paced tail iterations
                        for _ in range(2):
                            ps_f = pp.tile([128, 512], f32, tag="p")
                            nc.tensor.matmul(
                                ps_f,
                                kt_sb[:, 0, 0:128],
                                qt_sb[:, 0, 0:512],
                                start=True,
                                stop=True,
                            )
                    emit_scores_chunk(su, 2)
                    emit_scores_chunk(su, 3)
            # HAM warm-keepers: dead matmuls fill the wait for the last heads'
            # normalize chains so the output projection starts at full clock
            def warm(n):
                for _ in range(n):
                    ps_w = pss.tile([128, QL], f32, tag="s", name="warm")
                    nc.tensor.matmul(
                        ps_w[:, 0:512],
                        ctx_sb[:, 0, 0:128],
                        qt_sb[:, 0, 0:512],
                        start=True,
                        stop=True,
                    )

            warm(24)
            while pending_norm:
                pending_norm.pop(0)[1]()
                warm(4)

            # ================= output projection =================
            for qt in range(QL // 128):
                for jn in range(D // 512):
                    ps = pp.tile([128, 512], f32, tag="p")
                    for k in range(NT):
                        nc.tensor.matmul(
                            ps,
                            ctx_sb[:, k, qt * 128:(qt + 1) * 128],
                            wot_sb[:, k, jn * 512:(jn + 1) * 512],
                            start=(k == 0),
                            stop=(k == NT - 1),
                        )
                    o_sb = opool.tile([128, 512], f32, tag="o")
                    # split the copy+DMA of the final tile so the epilogue
                    # drain (last add -> last out DMA -> barrier) is shorter
                    nsp = 2 if (qt, jn) == (QL // 128 - 1, 1) else 1
                    for sp in range(nsp):
                        cs = slice(sp * 512 // nsp, (sp + 1) * 512 // nsp)
                        nc.vector.tensor_add(
                            o_sb[:, cs], ps[:, cs],
                            bob_sb[:, jn * 512 + cs.start:jn * 512 + cs.stop],
                        )
                        nc.sync.dma_start(
                            out=out_d[
                                qt * 128:(qt + 1) * 128,
                                jn * 512 + cs.start:jn * 512 + cs.stop,
                            ],
                            in_=o_sb[:, cs],
                        )

    nc.finalize()
    _NC_CACHE["nc"] = nc
    return nc


def _prep_in_maps(x, W_q, b_q, W_k, W_v, b_v, W_o, b_o):
    wqt = np.ascontiguousarray(W_q.T).astype(BF16)
    wkt = np.ascontiguousarray(W_k.T).astype(BF16)
    wot = np.ascontiguousarray(W_o.T).astype(BF16)
    # augmented W_v.T: per head 64 data columns + 1 zero column whose bias is
    # 1.0, so V gets a ones column and the context matmul also computes the
    # softmax denominator on psum partition 64
    wvt = np.zeros((D, DA), dtype=BF16)
    bvt = np.zeros((1, DA), dtype=np.float32)
    wv_t = np.asarray(W_v.T, dtype=np.float32)
    for h in range(H):
        wvt[:, h * 65:h * 65 + 64] = wv_t[:, h * 64:(h + 1) * 64].astype(BF16)
        bvt[0, h * 65:h * 65 + 64] = b_v[h * 64:(h + 1) * 64]
        bvt[0, h * 65 + 64] = 1.0
    bvt = bvt.astype(BF16)
    bq = np.ascontiguousarray(b_q.reshape(NT, 128).T).astype(np.float32)
    bot = b_o.reshape(1, D).astype(BF16)

    in_maps = []
    for c in range(8):
        b, qh = divmod(c, 2)
        xT = x[b].T  # [D, S]
        if qh == 0:
            xt = xT
        else:
            xt = np.concatenate([xT[:, QL:], xT[:, :QL]], axis=1)
        xt = np.ascontiguousarray(xt).astype(BF16)
        in_maps.append(
            {
                "xt": xt,
                "wqt": wqt, "wkt": wkt, "wvt": wvt, "wot": wot,
                "bq": bq, "bvt": bvt, "bot": bot,
            }
        )
    return in_maps


def _run(inputs, trace=False, trace_kwargs=None):
    from concourse import bass_utils

    nc = _build_nc()
    in_maps = _prep_in_maps(
        inputs["x"], inputs["W_q"], inputs["b_q"], inputs["W_k"],
        inputs["W_v"], inputs["b_v"], inputs["W_o"], inputs["b_o"],
    )
    kwargs = {}
    if trace:
        kwargs["trace"] = True
        if trace_kwargs:
            kwargs.update(trace_kwargs)
    res = bass_utils.run_bass_kernel_spmd(
        nc, in_maps, core_ids=list(range(8)), **kwargs
    )
    out = np.empty((4, S, D), np.float32)
    for c, r in enumerate(res.results):
        b, qh = divmod(c, 2)
        out[b, qh * QL:(qh + 1) * QL, :] = r["out"]
    return out, res


def kernel(**inputs):
    out, _ = _run(inputs, trace=False)
    return out
